# revision 13
# baseline (speedup 1.0000x reference)
"""Trainium2 Bass kernel for a 2-layer LSTM ODE integrator (BMEDLSTM).

Model (see harness reference): per time step t (T = int(max_time/0.1)+1 steps):
    inp   = concat([x, s])                       # [B, 80]
    h0,c0 = LSTMCell0(inp, (h0,c0))              # H=512
    h1,c1 = LSTMCell1(h0, (h1,c1))
    so    = h1 @ W_fc.T + b_fc                   # [B, 16]
    s     = s + 0.1 * so
Outputs: (outputs [B,T,16] = s history, cal_times [T], res_steps [B,T,16] = so history)

Strategy: data-parallel over batch across 8 NeuronCores (B=128 -> 16/core),
weights replicated. Per core, all matmuls run in "gates-transposed"
orientation: out[g_tile, b] = W_tile[k, g].T @ inp_T[k, b], with 128x128 bf16
stationary weight tiles (fast weight load) and the batch (16) as the moving
free dim. The elementwise LSTM cell runs in [128 gate-partitions, (m, b)]
layout and writes h_T tiles directly as next matmul inputs - no transposes,
no per-step DMA. Cell state c, integrator state s, and PSUM accumulation stay
fp32; matmul operands are bf16.

Layout bookkeeping:
 - gate columns permuted [i, f, o, g] (torch order is i, f, g, o)
 - L0 K-layout: [s(0:16); x(16:80); ones-row(80) -> folds b0; pad(81:128)] +
   h0_T (4 tiles); L1 K-layout: h0_T (4 tiles) + h1_T (4 tiles); b1 added via
   one DVE add of a precomputed broadcast tile; fc bias via tensor_scalar_add.
"""

import numpy as np
import ml_dtypes

import concourse.bass as bass
import concourse.mybir as mybir
from concourse.tile import TileContext
from concourse.bass_utils import run_bass_kernel_spmd

bf16 = ml_dtypes.bfloat16
fp8 = ml_dtypes.float8_e4m3
AF = mybir.ActivationFunctionType
ALU = mybir.AluOpType

import os

# Matmul operand dtype: "bf16" (2x fast weight load) or "fp8" (4x FWL, weights
# scaled by W_SCALE to escape the subnormal range, de-scaled for free in the
# activation's scale argument).
MM_DTYPE = os.environ.get("KERNEL_MM_DTYPE", "bf16")
W_SCALE = 1024.0 if MM_DTYPE == "fp8" else 1.0
# Column-tiled matmuls: split each 128x128 stationary tile into 4 [128,32]
# col-group tiles loaded concurrently into separate PE sub-arrays.
COLTILE = bool(int(os.environ.get("KERNEL_COLTILE", "0")))

N_CORES = 8
B_FULL = 128
B = B_FULL // N_CORES  # 16 per core
XS, SS, H = 64, 16, 512
G = 4 * H  # 2048
MT = G // 128  # 16 gate tiles per layer
DT_STEP = 0.1

# ---------------------------------------------------------------------------
# Workaround: this walrus build rejects instructions carrying more than one
# semaphore wait ("Too many sync wait commands"). After Tile scheduling,
# hoist excess waits onto NoOp instructions inserted just before the
# offending instruction on the same engine (same-engine program order makes
# this equivalent).
_MAX_WAITS = 1
_nop_counter = [0]


def _split_sync_waits(nc: "bass.Bass", limit: int = _MAX_WAITS):
    for fn in nc.m.functions:
        for bb in fn.blocks:
            insts = bb.instructions
            out = []
            changed = False
            for inst in insts:
                si = inst.sync_info
                waits = list(si.on_wait or []) if si is not None else []
                if len(waits) > limit and inst.engine is not None:
                    changed = True
                    keep = waits[-limit:]
                    rest = waits[: -limit]
                    while rest:
                        chunk, rest = rest[:limit], rest[limit:]
                        _nop_counter[0] += 1
                        nop = mybir.InstNoOp(
                            name=f"I-waitsplit-{_nop_counter[0]}",
                            engine=inst.engine,
                            ins=[],
                            outs=[],
                            sync_info=mybir.SyncInfo(on_wait=chunk, on_update=[]),
                        )
                        out.append(nop)
                    si.on_wait = keep
                out.append(inst)
            if changed:
                bb.instructions = out
# ---------------------------------------------------------------------------


def build_kernel(T: int, repeats: int = 1) -> bass.Bass:
    nc = bass.Bass("TRN2")
    f32 = mybir.dt.float32
    b16 = mybir.dt.float8e4 if MM_DTYPE == "fp8" else mybir.dt.bfloat16

    # DRAM I/O (per core)
    w0_d = nc.dram_tensor("w0", [128, 5, G], b16, kind="ExternalInput")
    w1_d = nc.dram_tensor("w1", [128, 8, G], b16, kind="ExternalInput")
    wfc_d = nc.dram_tensor("wfc", [128, 4, SS], b16, kind="ExternalInput")
    b1_d = nc.dram_tensor("b1f", [128, MT, B], f32, kind="ExternalInput")
    bfc_d = nc.dram_tensor("bfc", [SS, 1], f32, kind="ExternalInput")
    xT_d = nc.dram_tensor("xT", [XS, B], b16, kind="ExternalInput")
    s0T_d = nc.dram_tensor("s0T", [SS, B], b16, kind="ExternalInput")
    s0Tf_d = nc.dram_tensor("s0Tf", [SS, B], f32, kind="ExternalInput")
    outsT_d = nc.dram_tensor("outsT", [SS, T, B], f32, kind="ExternalOutput")
    resT_d = nc.dram_tensor("resT", [SS, T, B], f32, kind="ExternalOutput")

    with TileContext(nc) as tc:
        with (
            tc.tile_pool(name="weights", bufs=1) as wpool,
            tc.tile_pool(name="state", bufs=1) as spool,
            tc.tile_pool(name="hbuf", bufs=3) as hpool,
            tc.tile_pool(name="elt", bufs=3) as epool,
            tc.tile_pool(name="cbuf", bufs=3) as cpool,
            tc.tile_pool(name="psum", bufs=4, space="PSUM") as pspool,
            tc.tile_pool(name="psfc", bufs=2, space="PSUM") as pfcpool,
        ):
            # --- persistent tiles ---
            w0 = wpool.tile([128, 5, G], b16)
            w1 = wpool.tile([128, 8, G], b16)
            wfc = wpool.tile([128, 4, SS], b16)
            b1f = wpool.tile([128, MT, B], f32)
            bfc = wpool.tile([SS, 1], f32)
            inp0 = spool.tile([128, 1, B], b16)  # [s;x;1;pad] K-tile
            outbuf = spool.tile([SS, T, B], f32)  # s history (also s state)
            resbuf = spool.tile([SS, T, B], f32)  # step_out history
            s_init = spool.tile([SS, B], f32)

            nc.sync.dma_start(out=w0, in_=w0_d[:, :, :])
            nc.sync.dma_start(out=w1, in_=w1_d[:, :, :])
            nc.sync.dma_start(out=wfc, in_=wfc_d[:, :, :])
            nc.sync.dma_start(out=b1f, in_=b1_d[:, :, :])
            nc.sync.dma_start(out=bfc, in_=bfc_d[:, :])
            nc.vector.memset(inp0, 0.0)
            nc.sync.dma_start(out=inp0[0:SS, 0, :], in_=s0T_d[:, :])
            nc.sync.dma_start(out=inp0[SS : SS + XS, 0, :], in_=xT_d[:, :])
            nc.vector.memset(inp0[96:97, 0, :], 1.0)  # ones-row (32-aligned base)
            nc.sync.dma_start(out=s_init, in_=s0Tf_d[:, :])

            # initial h (bf16) and c (f32) are zero
            h0 = hpool.tile([128, 4, B], b16, tag="h0")
            h1 = hpool.tile([128, 4, B], b16, tag="h1")
            c0 = cpool.tile([128, 4, B], f32, tag="c0")
            c1 = cpool.tile([128, 4, B], f32, tag="c1")
            nc.vector.memset(h0, 0.0)
            nc.vector.memset(h1, 0.0)
            nc.vector.memset(c0, 0.0)
            nc.vector.memset(c1, 0.0)

            def lstm_layer(l, h_prev_a, h_prev_b, c_prev, htag, ctag):
                """Emit one LSTM layer; returns (h_new, c_new) tiles.

                l=0: rhs K-tiles = [inp0, h_prev_a(4)]     lhsT = w0
                l=1: rhs K-tiles = [h_prev_a(4), h_prev_b(4)] lhsT = w1
                """
                w = w0 if l == 0 else w1
                ps = pspool.tile([128, MT, B], f32, tag="ps")
                if l == 0:
                    rhs_tiles = [inp0[:, 0, :]] + [h_prev_a[:, k, :] for k in range(4)]
                else:
                    rhs_tiles = [h_prev_a[:, k, :] for k in range(4)] + [
                        h_prev_b[:, k, :] for k in range(4)
                    ]
                nk = len(rhs_tiles)
                for m in range(MT):
                    for k in range(nk):
                        if COLTILE:
                            for j in range(4):
                                nc.tensor.matmul(
                                    ps[32 * j : 32 * (j + 1), m, :],
                                    w[:, k, bass.ds(m * 128 + 32 * j, 32)],
                                    rhs_tiles[k],
                                    start=(k == 0),
                                    stop=(k == nk - 1),
                                    tile_position=(0, 32 * j),
                                )
                        else:
                            nc.tensor.matmul(
                                ps[:, m, :],
                                w[:, k, bass.ts(m, 128)],
                                rhs_tiles[k],
                                start=(k == 0),
                                stop=(k == nk - 1),
                            )
                if l == 1:
                    nc.vector.tensor_add(ps[:, :, :], ps[:, :, :], b1f[:, :, :])
                # sigmoid(i,f,o) tiles 0..11, tanh(g) tiles 12..15
                sig = epool.tile([128, 12, B], f32, tag="sig")
                tg = epool.tile([128, 4, B], f32, tag="tg")
                nc.scalar.activation(
                    out=sig, in_=ps[:, 0:12, :], func=AF.Sigmoid, scale=1.0 / W_SCALE
                )
                nc.scalar.activation(
                    out=tg, in_=ps[:, 12:16, :], func=AF.Tanh, scale=1.0 / W_SCALE
                )
                t1 = epool.tile([128, 4, B], f32, tag="t1")
                c_new = cpool.tile([128, 4, B], f32, tag=ctag)
                nc.vector.tensor_mul(t1, sig[:, 0:4, :], tg)  # i*tanh(g)
                nc.vector.tensor_mul(c_new, sig[:, 4:8, :], c_prev)  # f*c
                nc.vector.tensor_add(c_new, c_new, t1)
                tc_t = epool.tile([128, 4, B], f32, tag="tc")
                nc.scalar.activation(out=tc_t, in_=c_new, func=AF.Tanh)
                h_new = hpool.tile([128, 4, B], b16, tag=htag)
                nc.vector.tensor_mul(h_new, sig[:, 8:12, :], tc_t)  # o*tanh(c)
                return h_new, c_new

            for rep in range(repeats):
                for t in range(T):
                    h0, c0 = lstm_layer(0, h0, None, c0, "h0", "c0")
                    h1, c1 = lstm_layer(1, h0, h1, c1, "h1", "c1")
                    # fc: step_out.T [SS, B]
                    psf = pfcpool.tile([SS, B], f32, tag="psf")
                    for k in range(4):
                        nc.tensor.matmul(
                            psf,
                            wfc[:, k, :],
                            h1[:, k, :],
                            start=(k == 0),
                            stop=(k == 3),
                        )
                    nc.vector.tensor_scalar(
                        resbuf[:, t, :], psf, 1.0 / W_SCALE, bfc, ALU.mult, ALU.add
                    )
                    if t == 0:
                        s_prev = s_init if rep == 0 else outbuf[:, T - 1, :]
                    else:
                        s_prev = outbuf[:, t - 1, :]
                    nc.vector.scalar_tensor_tensor(
                        out=outbuf[:, t, :],
                        in0=resbuf[:, t, :],
                        scalar=DT_STEP,
                        in1=s_prev,
                        op0=ALU.mult,
                        op1=ALU.add,
                    )
                    nc.vector.tensor_copy(out=inp0[0:SS, 0, :], in_=outbuf[:, t, :])

            nc.sync.dma_start(out=outsT_d[:, :, :], in_=outbuf)
            nc.sync.dma_start(out=resT_d[:, :, :], in_=resbuf)

    _split_sync_waits(nc)
    return nc


# ---------------------------------------------------------------------------
# Host-side data prep


def _prep_shared(W_ih0, W_hh0, b_ih0, b_hh0, W_ih1, W_hh1, b_ih1, b_hh1, W_fc, b_fc):
    """Build the replicated weight arrays (bf16 tiles, gate-permuted)."""
    # torch gate order i,f,g,o -> ours i,f,o,g
    perm = np.concatenate(
        [np.arange(0, H), np.arange(H, 2 * H), np.arange(3 * H, 4 * H),
         np.arange(2 * H, 3 * H)]
    )

    # L0: K rows = [s(16); x(64); ones(1); pad(47); h0(512)] = 640 = 5*128
    W0T = np.zeros((640, G), np.float32)
    W0T[0:SS, :] = W_ih0[perm, XS : XS + SS].T
    W0T[SS : SS + XS, :] = W_ih0[perm, 0:XS].T
    W0T[96, :] = (b_ih0 + b_hh0)[perm]  # ones-row lives at partition 96
    W0T[128:640, :] = W_hh0[perm, :].T
    mmnp = fp8 if MM_DTYPE == "fp8" else bf16
    w0 = np.ascontiguousarray(
        (W0T * W_SCALE).reshape(5, 128, G).transpose(1, 0, 2)
    ).astype(mmnp)

    # L1: K rows = [h0(512); h1(512)] = 1024 = 8*128
    W1T = np.concatenate([W_ih1[perm, :].T, W_hh1[perm, :].T], axis=0).astype(
        np.float32
    )
    w1 = np.ascontiguousarray(
        (W1T * W_SCALE).reshape(8, 128, G).transpose(1, 0, 2)
    ).astype(mmnp)

    # fc: K rows = h1(512)
    WfcT = W_fc.T.astype(np.float32)  # [512, 16]
    wfc = np.ascontiguousarray(
        (WfcT * W_SCALE).reshape(4, 128, SS).transpose(1, 0, 2)
    ).astype(mmnp)

    b1p = ((b_ih1 + b_hh1)[perm] * W_SCALE).astype(np.float32)  # [2048]
    b1f = np.ascontiguousarray(
        np.broadcast_to(b1p.reshape(MT, 128).T[:, :, None], (128, MT, B))
    ).astype(np.float32)

    bfc = b_fc.reshape(SS, 1).astype(np.float32)
    return w0, w1, wfc, b1f, bfc


_BUILD_CACHE: dict[int, bass.Bass] = {}


def kernel(
    x,
    s0,
    W_ih0,
    W_hh0,
    b_ih0,
    b_hh0,
    W_ih1,
    W_hh1,
    b_ih1,
    b_hh1,
    W_fc,
    b_fc,
    max_time,
):
    x = np.asarray(x, np.float32)
    s0 = np.asarray(s0, np.float32)
    T = int(int(max_time) / DT_STEP) + 1

    w0, w1, wfc, b1f, bfc = _prep_shared(
        np.asarray(W_ih0, np.float32),
        np.asarray(W_hh0, np.float32),
        np.asarray(b_ih0, np.float32),
        np.asarray(b_hh0, np.float32),
        np.asarray(W_ih1, np.float32),
        np.asarray(W_hh1, np.float32),
        np.asarray(b_ih1, np.float32),
        np.asarray(b_hh1, np.float32),
        np.asarray(W_fc, np.float32),
        np.asarray(b_fc, np.float32),
    )

    if T not in _BUILD_CACHE:
        _BUILD_CACHE[T] = build_kernel(T)
    nc = _BUILD_CACHE[T]

    in_maps = []
    for core in range(N_CORES):
        xb = x[core * B : (core + 1) * B]  # [16, 64]
        sb = s0[core * B : (core + 1) * B]  # [16, 16]
        in_maps.append(
            {
                "w0": w0,
                "w1": w1,
                "wfc": wfc,
                "b1f": b1f,
                "bfc": bfc,
                "xT": np.ascontiguousarray(xb.T).astype(
                    fp8 if MM_DTYPE == "fp8" else bf16
                ),
                "s0T": np.ascontiguousarray(sb.T).astype(
                    fp8 if MM_DTYPE == "fp8" else bf16
                ),
                "s0Tf": np.ascontiguousarray(sb.T).astype(np.float32),
            }
        )

    import os

    trace = bool(int(os.environ.get("KERNEL_TRACE", "0")))
    res = run_bass_kernel_spmd(
        nc, in_maps, core_ids=list(range(N_CORES)), trace=trace
    )
    global LAST_RESULTS
    LAST_RESULTS = res

    outputs = np.empty((B_FULL, T, SS), np.float32)
    res_steps = np.empty((B_FULL, T, SS), np.float32)
    for core in range(N_CORES):
        r = res.results[core]
        outputs[core * B : (core + 1) * B] = r["outsT"].transpose(2, 1, 0)
        res_steps[core * B : (core + 1) * B] = r["resT"].transpose(2, 1, 0)
    cal_times = (np.arange(T) * DT_STEP).astype(np.float32)
    return outputs, cal_times, res_steps
